# revision 9
# baseline (speedup 1.0000x reference)
"""Multi-head cross-attention (b=2, n=m=2048, dim=1024, 16 heads) on 8 trn2 cores.

Sharding: core = be*4 + g  (be = batch element, g = head group of 4 heads).
Each core computes, for its batch element and its 4 heads:
    Q^T = (wq_g @ x1^T), K^T = (wk_g @ x2^T), V = x2 @ wv_g^T
    S^T = K^T.T @ Q^T  (per head), P = exp(S * scale), O^T = [V | 1].T @ P
    (ones column = softmax denominator), normalize via reciprocal +
    partition broadcast, y_partial = O @ wo_g^T.
Host sums the 4 head-group partials per batch element and adds the bias.

HW facts this version exploits (measured on this axon/trn2 setup):
  - matmul with contraction dim 64 runs ~2.3x slower per row than C=128
    (437 vs 171 ns per 512-row bf16 matmul). The per-head S^T matmul
    (contraction = head_dim = 64) therefore uses the PACKED two-head K^T
    stationary (128 partitions) against a zero-padded per-head Q^T moving
    operand (the head's 64 rows hold Q^T, the other 64 rows are zero),
    computing the same S^T at full C=128 rate.
  - bf16 matmuls are ~1.27x faster than fp32r and halve SBUF/DMA traffic.
  - Engine queues execute IN ORDER, so emission order is the schedule.
    The exp stream (ACT, ~147us total) must never be starved: S^T pairs,
    the previous group's O accumulation, and out/Q-projection work are
    interleaved at ~1 slab granularity so no PE run between S-pairs
    exceeds the ~1.15us exp slab time.
"""

import sys

if "/opt/trn_rl_repo" not in sys.path:
    sys.path.insert(0, "/opt/trn_rl_repo")

import numpy as np

import concourse.tile as tile
from concourse import bacc, mybir
from concourse import bass_utils

P = 128
NTOK = 2048            # n = m = token count per batch element
DIM = 1024
HPC = 4                # heads per core
DH = 64                # head dim
HD = HPC * DH          # 256 = per-core projection width
ECH = DIM // P         # 8 contraction chunks
NCH = NTOK // 512      # 4 n-chunks of 512
MT = NTOK // P         # 16 m-tiles of 128
NG = NCH * HPC         # 16 (nq, h) groups
SCALE = DH ** -0.5
F32 = mybir.dt.float32
BF16 = mybir.dt.bfloat16

_CACHE: dict = {}


def _build(trace_sim: bool = False, repeat: int = 1):
    EXP = mybir.ActivationFunctionType.Exp
    nc = bacc.Bacc("TRN2", target_bir_lowering=False, debug=False, num_devices=8)
    x1T = nc.dram_tensor("x1t", [DIM, NTOK], BF16, kind="ExternalInput").ap()
    x2T = nc.dram_tensor("x2t", [DIM, NTOK], BF16, kind="ExternalInput").ap()
    wqT = nc.dram_tensor("wqt", [DIM, HD], BF16, kind="ExternalInput").ap()
    wkT = nc.dram_tensor("wkt", [DIM, HD], BF16, kind="ExternalInput").ap()
    wvT = nc.dram_tensor("wvt", [DIM, HD], BF16, kind="ExternalInput").ap()
    woT = nc.dram_tensor("wot", [HD, DIM], BF16, kind="ExternalInput").ap()
    y = nc.dram_tensor("y", [NTOK, DIM], BF16, kind="ExternalOutput").ap()

    x1T_s = x1T.rearrange("(po pi) n -> pi po n", pi=P)      # [128, 8, 2048]
    x2T_s = x2T.rearrange("(po pi) n -> pi po n", pi=P)
    wqT_r = wqT.rearrange("(po pi) m -> pi po m", pi=P)      # [128, 8, 256]
    wkT_r = wkT.rearrange("(po pi) m -> pi po m", pi=P)
    wvT_r = wvT.rearrange("(po pi) m -> pi po m", pi=P)
    woT_r = woT.rearrange("(po pi) e -> pi po e", pi=P)      # [128, 2, 1024]
    scr = [
        nc.dram_tensor(f"scr{r}", [NG, 2, 512], F32, kind="Internal").ap()
        for r in range(repeat)
    ]

    with tile.TileContext(nc, trace_sim=trace_sim) as tc:
      for _rep in range(repeat):
        with (
            tc.tile_pool(name="persist", bufs=1) as persist,
            tc.tile_pool(name="psO1", bufs=1, space="PSUM") as psOp,  # [128,512]
            tc.tile_pool(name="prj", bufs=1, space="PSUM") as prjp,   # [128,512]
            tc.tile_pool(name="psS", bufs=3, space="PSUM") as psSp,  # [128,1024]
            tc.tile_pool(name="xq", bufs=2) as xqpool,
            tc.tile_pool(name="slab", bufs=24) as slabpool,
            tc.tile_pool(name="rec", bufs=2) as recpool,
            tc.tile_pool(name="bcp", bufs=2) as bcpool,
            tc.tile_pool(name="otmp", bufs=2) as tmppool,
            tc.tile_pool(name="ysb", bufs=2) as ypool,
        ):
            wq_sb = persist.tile([P, ECH, HD], BF16, tag="wq")
            wo_sb = persist.tile([P, 2, DIM], BF16, tag="wo")
            onesf = persist.tile([P, 64], BF16, tag="onesf")
            nc.vector.memset(onesf[:], 1.0)
            # warm the ACT exp table during initial DMAs
            dum = persist.tile([P, 8], F32, tag="dum")
            nc.vector.memset(dum[:], 0.0)
            nc.scalar.activation(dum[:], dum[:], EXP)
            # zero-padded per-head Q^T: head h occupies rows 64*(h%2)..+64 of
            # QTz[:, h, :]; the other 64 rows stay zero so the S^T matmul can
            # use the packed two-head K^T stationary at C=128.
            QTz = persist.tile([P, HPC, NTOK], BF16, tag="QTz")
            nc.vector.memset(QTz[:], 0.0)
            O_sb = persist.tile([P, 2, NTOK], BF16, tag="O")
            KT_sb = persist.tile([P, 2, NTOK], BF16, tag="KT")
            V_sb = persist.tile([P, MT, HPC, 65], BF16, tag="V")
            nc.vector.tensor_copy(
                V_sb[:, :, :, 64:65],
                onesf[:].rearrange("p (a b c) -> p a b c", a=MT, b=HPC, c=1),
            )

            slabs = {}

            def s_exp_pair(g, mtp):
                # S^T + exp for m-tiles (2*mtp, 2*mtp+1) of group g=(nq,h)
                nq, h = g // HPC, g % HPC
                pg = h // 2
                ns = slice(nq * 512, (nq + 1) * 512)
                psS = psSp.tile([P, 1024], F32, tag="psS", name=f"psS{g}_{mtp}")
                for sub in range(2):
                    mt = 2 * mtp + sub
                    nc.tensor.matmul(
                        psS[:, sub * 512:(sub + 1) * 512],
                        KT_sb[:, pg, mt * P:(mt + 1) * P],
                        QTz[:, h, ns],
                        start=True,
                        stop=True,
                    )
                es = slabpool.tile([P, 1024], BF16, tag="es", name=f"es{g}_{mtp}")
                slabs[(g, mtp)] = es
                nc.scalar.activation(es[:], psS[:], EXP, scale=SCALE)

            psO_t = {}

            def o_pair(g, mtp):
                # two O^T accumulation steps for group g (m-tiles 2mtp,2mtp+1)
                h = g % HPC
                if mtp == 0:
                    psO_t[g] = psOp.tile([P, 512], F32, tag="po", name=f"psO{g}")
                psO = psO_t[g]
                es = slabs[(g, mtp)]
                for sub in range(2):
                    mt = 2 * mtp + sub
                    nc.tensor.matmul(
                        psO[0:65, :],
                        V_sb[:, mt, h, :],
                        es[:, sub * 512:(sub + 1) * 512],
                        start=(mt == 0),
                        stop=(mt == MT - 1),
                    )

            def normalize(g):
                nq, h = g // HPC, g % HPC
                pg, pos = h // 2, h % 2
                ns = slice(nq * 512, (nq + 1) * 512)
                psO = psO_t.pop(g)
                oc = recpool.tile([P, 512], F32, tag="rec")
                nc.vector.tensor_copy(oc[0:65, :], psO[0:65, :])
                # reciprocal of the 512 denominators: spread them over 64
                # partitions first so the ~6 cycle/elem DVE reciprocal costs
                # ~0.2us instead of 3.2us on one partition (it would head-of-
                # line block the in-order DVE queue).
                # bounce the [1,512] denom row through DRAM to respread it
                # over 64 partitions (SBUF APs cannot re-partition in place)
                nc.sync.dma_start(scr[_rep][g, 0], oc[64:65, :])
                rcp = bcpool.tile([64, 16], F32, tag="rcp")
                nc.sync.dma_start(rcp[:, 0:8], scr[_rep][g, 0])
                with nc.allow_low_precision(reason="softmax denom reciprocal"):
                    nc.vector.reciprocal(rcp[:, 8:16], rcp[:, 0:8])
                nc.sync.dma_start(scr[_rep][g, 1], rcp[:, 8:16])
                bcs = bcpool.tile([64, 512], F32, tag="bcs")
                nc.sync.dma_start(bcs[0:1, :], scr[_rep][g, 1])
                bc = bcpool.tile([64, 512], F32, tag="bc")
                nc.gpsimd.partition_broadcast(bc[:], bcs[0:1, :])
                if pos == 0:
                    nc.gpsimd.tensor_mul(O_sb[0:64, pg, ns], oc[0:64, :], bc[:])
                else:
                    tmp = tmppool.tile([64, 512], BF16, tag="otmp")
                    nc.gpsimd.tensor_mul(tmp[:], oc[0:64, :], bc[:])
                    nc.sync.dma_start(O_sb[64:128, pg, ns], tmp[:])

            # --- small PE work pieces, popped between slabs -----------------
            extra_q = []

            def qproj_pieces(nq):
                # Q^T projection for n-chunk nq, split into ~2-matmul pieces
                ns = slice(nq * 512, (nq + 1) * 512)
                state = {}

                def dma_piece():
                    xq = xqpool.tile([P, ECH, 512], BF16, tag="xq")
                    state["xq"] = xq
                    for ec in range(ECH):
                        nc.sync.dma_start(xq[:, ec], x1T_s[:, ec, ns])

                def mm_piece(pg, e0):
                    def run():
                        if e0 == 0:
                            state[pg] = prjp.tile(
                                [P, 512], F32, tag="pj", name=f"psq{nq}{pg}"
                            )
                        psq = state[pg]
                        for ec in (e0, e0 + 1):
                            nc.tensor.matmul(
                                psq[:],
                                wq_sb[:, ec, pg * P:(pg + 1) * P],
                                state["xq"][:, ec, :],
                                start=(ec == 0),
                                stop=(ec == ECH - 1),
                            )
                        if e0 == ECH - 2:
                            nc.vector.tensor_copy(
                                QTz[0:64, 2 * pg, ns], psq[0:64, :]
                            )
                            nc.vector.tensor_copy(
                                QTz[64:128, 2 * pg + 1, ns], psq[64:128, :]
                            )
                    return run

                yield dma_piece
                for pg in range(2):
                    for e0 in range(0, ECH, 2):
                        yield mm_piece(pg, e0)

            def outproj_pieces(nq):
                state = {}

                def mm_piece(nt, eo):
                    def run():
                        if eo == 0:
                            state[nt] = ypool.tile(
                                [P, DIM], BF16, tag="y", name=f"ysb{nt}"
                            )
                        psY = prjp.tile([P, 512], F32, tag="pj", name=f"psY{nt}{eo}")
                        for pg in range(2):
                            nc.tensor.matmul(
                                psY[:],
                                O_sb[:, pg, nt * P:(nt + 1) * P],
                                wo_sb[:, pg, eo * 512:(eo + 1) * 512],
                                start=(pg == 0),
                                stop=(pg == 1),
                            )
                        nc.vector.tensor_copy(
                            state[nt][:, eo * 512:(eo + 1) * 512], psY[:]
                        )
                        if eo == 1:
                            nc.gpsimd.dma_start(
                                y[nt * P:(nt + 1) * P, :], state[nt][:]
                            )
                    return run

                for nt in range(4 * nq, 4 * nq + 4):
                    for eo in range(2):
                        yield mm_piece(nt, eo)

            def pop_extra(k=1):
                for _ in range(k):
                    if extra_q:
                        extra_q.pop(0)()

            # ---- warmup: single x2 pass: K^T + V projections, with groups
            # 0 and 1 (nq=0, h=0/1) S^T+exp emitted as K chunks land ----
            with (
                tc.tile_pool(name="wkv", bufs=1) as wkvpool,
                tc.tile_pool(name="xk", bufs=2) as xkpool,
            ):
                wk_sb = wkvpool.tile([P, ECH, HD], BF16, tag="wk")
                for ec in range(ECH):
                    nc.sync.dma_start(wk_sb[:, ec], wkT_r[:, ec])
                wv_sb = wkvpool.tile([P, ECH, HD], BF16, tag="wv")

                for nq in range(NCH):
                    ns = slice(nq * 512, (nq + 1) * 512)
                    xk = xkpool.tile([P, ECH, 512], BF16, tag="xk")
                    for ec in range(ECH):
                        nc.sync.dma_start(xk[:, ec], x2T_s[:, ec, ns])
                    if nq == 0:
                        for ec in range(ECH):
                            nc.sync.dma_start(wv_sb[:, ec], wvT_r[:, ec])
                    for pg in range(2):
                        pl, tg = (psOp, "po") if pg % 2 == 0 else (prjp, "pj")
                        psq = pl.tile([P, 512], F32, tag=tg, name=f"psk{nq}{pg}")
                        for ec in range(ECH):
                            nc.tensor.matmul(
                                psq[:],
                                wk_sb[:, ec, pg * P:(pg + 1) * P],
                                xk[:, ec, :],
                                start=(ec == 0),
                                stop=(ec == ECH - 1),
                            )
                        nc.vector.tensor_copy(KT_sb[:, pg, ns], psq[:])
                    # V for the 4 m-tiles covered by this x2 chunk
                    for sub in range(4):
                        mt = 4 * nq + sub
                        pl, tg = (psOp, "po") if sub % 2 == 0 else (prjp, "pj")
                        pv = pl.tile([P, 512], F32, tag=tg, name=f"psv{mt}")
                        for ec in range(ECH):
                            nc.tensor.matmul(
                                pv[:, 0:256],
                                xk[:, ec, sub * P:(sub + 1) * P],
                                wv_sb[:, ec, :],
                                start=(ec == 0),
                                stop=(ec == ECH - 1),
                            )
                        nc.vector.tensor_copy(
                            V_sb[:, mt, :, 0:64],
                            pv[:, 0:256].rearrange("p (h d) -> p h d", d=64),
                        )
                    if nq == 0:
                        nc.sync.dma_start(wq_sb[:], wqT_r)
                        for piece in qproj_pieces(0):
                            piece()
                    if nq == NCH - 1:
                        nc.sync.dma_start(wo_sb[:], woT_r)
                    # warmup S/exp: groups 0,1 (h=0,1 need only pg=0 K rows,
                    # both live in the packed KT chunk just produced)
                    for h in range(2):
                        for mtp in (2 * nq, 2 * nq + 1):
                            s_exp_pair(h, mtp)

            # ---- main software pipeline over groups: S(idx) slabs are
            # interleaved with O(idx-1) pairs and small projection pieces ----
            for idx in range(1, NG):
                nq, h = idx // HPC, idx % HPC
                if h == 1 and nq + 1 < NCH:
                    extra_q.extend(qproj_pieces(nq + 1))
                if h == 2 and nq >= 1:
                    extra_q.extend(outproj_pieces(nq - 1))
                for mtp in range(MT // 2):
                    if idx >= 2:
                        s_exp_pair(idx, mtp)
                    o_pair(idx - 1, mtp)
                    pop_extra(1)
                normalize(idx - 1)
            for mtp in range(MT // 2):
                o_pair(NG - 1, mtp)
                pop_extra(2)
            normalize(NG - 1)
            while extra_q:
                pop_extra(1)
            for piece in outproj_pieces(NCH - 1):
                piece()
    nc.compile()
    return nc


def get_nc(trace_sim: bool = False, repeat: int = 1):
    key = ("nc", trace_sim, repeat)
    if key not in _CACHE:
        _CACHE[key] = _build(trace_sim, repeat)
    return _CACHE[key]


def make_in_maps(x1, x2, wq, wk, wv, wo):
    import ml_dtypes
    bf16 = ml_dtypes.bfloat16
    x1 = np.asarray(x1, dtype=np.float32)
    x2 = np.asarray(x2, dtype=np.float32)
    wq = np.asarray(wq, dtype=np.float32)
    wk = np.asarray(wk, dtype=np.float32)
    wv = np.asarray(wv, dtype=np.float32)
    wo = np.asarray(wo, dtype=np.float32)
    in_maps = []
    for core in range(8):
        be, g = core // 4, core % 4
        sl = slice(HD * g, HD * (g + 1))
        in_maps.append({
            "x1t": np.ascontiguousarray(x1[be].T).astype(bf16),
            "x2t": np.ascontiguousarray(x2[be].T).astype(bf16),
            "wqt": np.ascontiguousarray(wq[sl, :].T).astype(bf16),
            "wkt": np.ascontiguousarray(wk[sl, :].T).astype(bf16),
            "wvt": np.ascontiguousarray(wv[sl, :].T).astype(bf16),
            "wot": np.ascontiguousarray(wo[:, sl].T).astype(bf16),
        })
    return in_maps


def assemble(results, bo):
    bo = np.asarray(bo, dtype=np.float32)
    out = np.empty((2, NTOK, DIM), np.float32)
    for be in range(2):
        acc = results[be * 4]["y"].astype(np.float32)
        for g in range(1, 4):
            acc += results[be * 4 + g]["y"].astype(np.float32)
        out[be] = acc + bo
    return out


def kernel(x1, x2, wq, wk, wv, wo, bo):
    nc = get_nc()
    in_maps = make_in_maps(x1, x2, wq, wk, wv, wo)
    last_err = None
    for attempt in range(3):
        try:
            res = bass_utils.run_bass_kernel_spmd(
                nc, in_maps, core_ids=list(range(8))
            )
            return assemble(res.results, bo)
        except Exception as e:  # transient NRT_EXEC_UNIT_UNRECOVERABLE etc.
            last_err = e
            import time as _time
            _time.sleep(5 * (attempt + 1))
    raise last_err


# revision 10
# speedup vs baseline: 1.4606x; 1.4606x over previous
"""Multi-head cross-attention (b=2, n=m=2048, dim=1024, 16 heads) on 8 trn2 cores.

Sharding: core = be*4 + g  (be = batch element, g = head group of 4 heads).
Each core computes, for its batch element and its 4 heads:
    Q^T = (wq_g @ x1^T), K^T = (wk_g @ x2^T), V = x2 @ wv_g^T
    S^T = K^T.T @ Q^T  (per head), P = exp(S * scale), O^T = [V | 1].T @ P
    (ones column = softmax denominator), normalize via reciprocal +
    partition broadcast, y_partial = O @ wo_g^T.
Host sums the 4 head-group partials per batch element and adds the bias.

HW facts this version exploits (measured on this axon/trn2 setup):
  - matmul with contraction dim 64 runs ~2.3x slower per row than C=128
    (437 vs 171 ns per 512-row bf16 matmul). The per-head S^T matmul
    (contraction = head_dim = 64) therefore uses the PACKED two-head K^T
    stationary (128 partitions) against a zero-padded per-head Q^T moving
    operand (the head's 64 rows hold Q^T, the other 64 rows are zero),
    computing the same S^T at full C=128 rate.
  - bf16 matmuls are ~1.27x faster than fp32r and halve SBUF/DMA traffic.
  - Engine queues execute IN ORDER, so emission order is the schedule.
    S^T pairs, the previous group's O accumulation, and out/Q-projection
    work are interleaved at ~1 slab granularity so no PE run between
    S-pairs exceeds the ~1us exp slab time.
  - The sync/SP queue (tile alloc/release bookkeeping + DMA issue) is a
    hidden serial resource: exp-output tiles are allocated 4 slabs at a
    time and x/weight loads are single strided DMAs to keep SP short.
"""

import sys

if "/opt/trn_rl_repo" not in sys.path:
    sys.path.insert(0, "/opt/trn_rl_repo")

import numpy as np

import concourse.tile as tile
from concourse import bacc, mybir
from concourse import bass_utils

P = 128
NTOK = 2048            # n = m = token count per batch element
DIM = 1024
HPC = 4                # heads per core
DH = 64                # head dim
HD = HPC * DH          # 256 = per-core projection width
ECH = DIM // P         # 8 contraction chunks
NCH = NTOK // 512      # 4 n-chunks of 512
MT = NTOK // P         # 16 m-tiles of 128
NG = NCH * HPC         # 16 (nq, h) groups
SCALE = DH ** -0.5
F32 = mybir.dt.float32
BF16 = mybir.dt.bfloat16

_CACHE: dict = {}


def _build(trace_sim: bool = False, repeat: int = 1):
    EXP = mybir.ActivationFunctionType.Exp
    nc = bacc.Bacc("TRN2", target_bir_lowering=False, debug=False, num_devices=8)
    x1T = nc.dram_tensor("x1t", [DIM, NTOK], BF16, kind="ExternalInput").ap()
    x2T = nc.dram_tensor("x2t", [DIM, NTOK], BF16, kind="ExternalInput").ap()
    wqT = nc.dram_tensor("wqt", [DIM, HD], BF16, kind="ExternalInput").ap()
    wkT = nc.dram_tensor("wkt", [DIM, HD], BF16, kind="ExternalInput").ap()
    wvT = nc.dram_tensor("wvt", [DIM, HD], BF16, kind="ExternalInput").ap()
    woT = nc.dram_tensor("wot", [HD, DIM], BF16, kind="ExternalInput").ap()
    y = nc.dram_tensor("y", [NTOK, DIM], BF16, kind="ExternalOutput").ap()

    x1T_s = x1T.rearrange("(po pi) n -> pi po n", pi=P)      # [128, 8, 2048]
    x2T_s = x2T.rearrange("(po pi) n -> pi po n", pi=P)
    wqT_r = wqT.rearrange("(po pi) m -> pi po m", pi=P)      # [128, 8, 256]
    wkT_r = wkT.rearrange("(po pi) m -> pi po m", pi=P)
    wvT_r = wvT.rearrange("(po pi) m -> pi po m", pi=P)
    woT_r = woT.rearrange("(po pi) e -> pi po e", pi=P)      # [128, 2, 1024]

    with tile.TileContext(nc, trace_sim=trace_sim) as tc:
      for _rep in range(repeat):
        with (
            tc.tile_pool(name="persist", bufs=1) as persist,
            tc.tile_pool(name="ps1", bufs=4, space="PSUM") as ps1,   # [128,512]
            tc.tile_pool(name="psS", bufs=2, space="PSUM") as psSp,  # [128,1024]
            tc.tile_pool(name="xq", bufs=2) as xqpool,
            tc.tile_pool(name="slab", bufs=6) as slabpool,
            tc.tile_pool(name="rec", bufs=2) as recpool,
            tc.tile_pool(name="bcp", bufs=2) as bcpool,
            tc.tile_pool(name="otmp", bufs=2) as tmppool,
            tc.tile_pool(name="ysb", bufs=2) as ypool,
        ):
            wq_sb = persist.tile([P, ECH, HD], BF16, tag="wq")
            wo_sb = persist.tile([P, 2, DIM], BF16, tag="wo")
            onesf = persist.tile([P, 64], BF16, tag="onesf")
            nc.vector.memset(onesf[:], 1.0)
            # warm the ACT exp table during initial DMAs
            dum = persist.tile([P, 8], F32, tag="dum")
            nc.vector.memset(dum[:], 0.0)
            nc.scalar.activation(dum[:], dum[:], EXP)
            # zero-padded per-head Q^T: head h occupies rows 64*(h%2)..+64 of
            # QTz[:, h, :]; the other 64 rows stay zero so the S^T matmul can
            # use the packed two-head K^T stationary at C=128.
            QTz = persist.tile([P, HPC, NTOK], BF16, tag="QTz")
            nc.vector.memset(QTz[:], 0.0)
            O_sb = persist.tile([P, 2, NTOK], BF16, tag="O")
            KT_sb = persist.tile([P, 2, NTOK], BF16, tag="KT")
            V_sb = persist.tile([P, MT, HPC, 65], BF16, tag="V")
            nc.vector.tensor_copy(
                V_sb[:, :, :, 64:65],
                onesf[:].rearrange("p (a b c) -> p a b c", a=MT, b=HPC, c=1),
            )

            slabs = {}

            def s_exp_pair(g, mtp):
                # S^T + exp for m-tiles (2*mtp, 2*mtp+1) of group g=(nq,h)
                nq, h = g // HPC, g % HPC
                pg = h // 2
                ns = slice(nq * 512, (nq + 1) * 512)
                psS = psSp.tile([P, 1024], F32, tag="psS", name=f"psS{g}_{mtp}")
                for sub in range(2):
                    mt = 2 * mtp + sub
                    nc.tensor.matmul(
                        psS[:, sub * 512:(sub + 1) * 512],
                        KT_sb[:, pg, mt * P:(mt + 1) * P],
                        QTz[:, h, ns],
                        start=True,
                        stop=True,
                    )
                # exp-output tiles cover 4 slabs each: fewer pool alloc /
                # release meta-ops on the serial sync queue
                if mtp % 4 == 0:
                    slabs[(g, mtp // 4)] = slabpool.tile(
                        [P, 4, 1024], BF16, tag="es", name=f"es{g}_{mtp // 4}"
                    )
                es = slabs[(g, mtp // 4)]
                nc.scalar.activation(es[:, mtp % 4], psS[:], EXP, scale=SCALE)

            psO_t = {}

            def o_pair(g, mtp):
                # two O^T accumulation steps for group g (m-tiles 2mtp,2mtp+1)
                h = g % HPC
                if mtp == 0:
                    psO_t[g] = ps1.tile([P, 512], F32, tag="b1", name=f"psO{g}")
                psO = psO_t[g]
                es = slabs[(g, mtp // 4)]
                for sub in range(2):
                    mt = 2 * mtp + sub
                    nc.tensor.matmul(
                        psO[0:65, :],
                        V_sb[:, mt, h, :],
                        es[:, mtp % 4, sub * 512:(sub + 1) * 512],
                        start=(mt == 0),
                        stop=(mt == MT - 1),
                    )

            def normalize(g):
                nq, h = g // HPC, g % HPC
                pg, pos = h // 2, h % 2
                ns = slice(nq * 512, (nq + 1) * 512)
                psO = psO_t.pop(g)
                rec = recpool.tile([P, 512], F32, tag="rec")
                with nc.allow_low_precision(reason="softmax denom reciprocal"):
                    nc.vector.reciprocal(rec[64:65, :], psO[64:65, :])
                nc.gpsimd.dma_start(rec[0:1, :], rec[64:65, :])
                bc = bcpool.tile([64, 512], F32, tag="bc")
                nc.gpsimd.partition_broadcast(bc[:], rec[0:1, :])
                if pos == 0:
                    nc.vector.tensor_mul(O_sb[0:64, pg, ns], psO[0:64, :], bc[:])
                else:
                    tmp = tmppool.tile([64, 512], BF16, tag="otmp")
                    nc.vector.tensor_mul(tmp[:], psO[0:64, :], bc[:])
                    nc.gpsimd.dma_start(O_sb[64:128, pg, ns], tmp[:])

            # --- small PE work pieces, popped between slabs -----------------
            extra_q = []

            def qproj_pieces(nq):
                # Q^T projection for n-chunk nq, split into ~2-matmul pieces
                ns = slice(nq * 512, (nq + 1) * 512)
                state = {}

                def dma_piece():
                    xq = xqpool.tile([P, ECH, 512], BF16, tag="xq")
                    state["xq"] = xq
                    nc.sync.dma_start(xq[:], x1T_s[:, :, ns])

                def mm_piece(pg, e0):
                    def run():
                        if e0 == 0:
                            state[pg] = ps1.tile(
                                [P, 512], F32, tag="b1", name=f"psq{nq}{pg}"
                            )
                        psq = state[pg]
                        for ec in (e0, e0 + 1):
                            nc.tensor.matmul(
                                psq[:],
                                wq_sb[:, ec, pg * P:(pg + 1) * P],
                                state["xq"][:, ec, :],
                                start=(ec == 0),
                                stop=(ec == ECH - 1),
                            )
                        if e0 == ECH - 2:
                            nc.vector.tensor_copy(
                                QTz[0:64, 2 * pg, ns], psq[0:64, :]
                            )
                            nc.vector.tensor_copy(
                                QTz[64:128, 2 * pg + 1, ns], psq[64:128, :]
                            )
                    return run

                yield dma_piece
                for pg in range(2):
                    for e0 in range(0, ECH, 2):
                        yield mm_piece(pg, e0)

            def outproj_pieces(nq):
                state = {}

                def mm_piece(nt, eo):
                    def run():
                        if eo == 0:
                            state[nt] = ypool.tile(
                                [P, DIM], BF16, tag="y", name=f"ysb{nt}"
                            )
                        psY = ps1.tile([P, 512], F32, tag="b1", name=f"psY{nt}{eo}")
                        for pg in range(2):
                            nc.tensor.matmul(
                                psY[:],
                                O_sb[:, pg, nt * P:(nt + 1) * P],
                                wo_sb[:, pg, eo * 512:(eo + 1) * 512],
                                start=(pg == 0),
                                stop=(pg == 1),
                            )
                        nc.vector.tensor_copy(
                            state[nt][:, eo * 512:(eo + 1) * 512], psY[:]
                        )
                        if eo == 1:
                            nc.gpsimd.dma_start(
                                y[nt * P:(nt + 1) * P, :], state[nt][:]
                            )
                    return run

                for nt in range(4 * nq, 4 * nq + 4):
                    for eo in range(2):
                        yield mm_piece(nt, eo)

            def pop_extra(k=1):
                for _ in range(k):
                    if extra_q:
                        extra_q.pop(0)()

            # ---- warmup: single x2 pass: K^T + V projections, with groups
            # 0 and 1 (nq=0, h=0/1) S^T+exp emitted as K chunks land ----
            with (
                tc.tile_pool(name="wkv", bufs=1) as wkvpool,
                tc.tile_pool(name="xk", bufs=2) as xkpool,
            ):
                wk_sb = wkvpool.tile([P, ECH, HD], BF16, tag="wk")
                nc.sync.dma_start(wk_sb[:], wkT_r)
                wv_sb = wkvpool.tile([P, ECH, HD], BF16, tag="wv")

                for nq in range(NCH):
                    ns = slice(nq * 512, (nq + 1) * 512)
                    xk = xkpool.tile([P, ECH, 512], BF16, tag="xk")
                    nc.sync.dma_start(xk[:], x2T_s[:, :, ns])
                    if nq == 0:
                        nc.sync.dma_start(wv_sb[:], wvT_r)
                    for pg in range(2):
                        psq = ps1.tile([P, 512], F32, tag="b1", name=f"psk{nq}{pg}")
                        for ec in range(ECH):
                            nc.tensor.matmul(
                                psq[:],
                                wk_sb[:, ec, pg * P:(pg + 1) * P],
                                xk[:, ec, :],
                                start=(ec == 0),
                                stop=(ec == ECH - 1),
                            )
                        nc.vector.tensor_copy(KT_sb[:, pg, ns], psq[:])
                    # V for the 4 m-tiles covered by this x2 chunk
                    for sub in range(4):
                        mt = 4 * nq + sub
                        pv = ps1.tile([P, 512], F32, tag="b1", name=f"psv{mt}")
                        for ec in range(ECH):
                            nc.tensor.matmul(
                                pv[:, 0:256],
                                xk[:, ec, sub * P:(sub + 1) * P],
                                wv_sb[:, ec, :],
                                start=(ec == 0),
                                stop=(ec == ECH - 1),
                            )
                        nc.vector.tensor_copy(
                            V_sb[:, mt, :, 0:64],
                            pv[:, 0:256].rearrange("p (h d) -> p h d", d=64),
                        )
                    if nq == 0:
                        nc.sync.dma_start(wq_sb[:], wqT_r)
                        for piece in qproj_pieces(0):
                            piece()
                    if nq == NCH - 1:
                        nc.sync.dma_start(wo_sb[:], woT_r)
                    # warmup S/exp: groups 0,1 over the m-tiles this K chunk
                    # just produced
                    for h in range(2):
                        for mtp in (2 * nq, 2 * nq + 1):
                            s_exp_pair(h, mtp)

            # ---- main software pipeline over groups: S(idx) slabs are
            # interleaved with O(idx-1) pairs and small projection pieces ----
            for idx in range(1, NG):
                nq, h = idx // HPC, idx % HPC
                if h == 1 and nq + 1 < NCH:
                    extra_q.extend(qproj_pieces(nq + 1))
                if h == 2 and nq >= 1:
                    extra_q.extend(outproj_pieces(nq - 1))
                for mtp in range(MT // 2):
                    if idx >= 2:
                        s_exp_pair(idx, mtp)
                    o_pair(idx - 1, mtp)
                    pop_extra(1)
                normalize(idx - 1)
            for mtp in range(MT // 2):
                o_pair(NG - 1, mtp)
                pop_extra(2)
            normalize(NG - 1)
            while extra_q:
                pop_extra(1)
            for piece in outproj_pieces(NCH - 1):
                piece()
    nc.compile()
    return nc


def get_nc(trace_sim: bool = False, repeat: int = 1):
    key = ("nc", trace_sim, repeat)
    if key not in _CACHE:
        _CACHE[key] = _build(trace_sim, repeat)
    return _CACHE[key]


def make_in_maps(x1, x2, wq, wk, wv, wo):
    import ml_dtypes
    bf16 = ml_dtypes.bfloat16
    x1 = np.asarray(x1, dtype=np.float32)
    x2 = np.asarray(x2, dtype=np.float32)
    wq = np.asarray(wq, dtype=np.float32)
    wk = np.asarray(wk, dtype=np.float32)
    wv = np.asarray(wv, dtype=np.float32)
    wo = np.asarray(wo, dtype=np.float32)
    in_maps = []
    for core in range(8):
        be, g = core // 4, core % 4
        sl = slice(HD * g, HD * (g + 1))
        in_maps.append({
            "x1t": np.ascontiguousarray(x1[be].T).astype(bf16),
            "x2t": np.ascontiguousarray(x2[be].T).astype(bf16),
            "wqt": np.ascontiguousarray(wq[sl, :].T).astype(bf16),
            "wkt": np.ascontiguousarray(wk[sl, :].T).astype(bf16),
            "wvt": np.ascontiguousarray(wv[sl, :].T).astype(bf16),
            "wot": np.ascontiguousarray(wo[:, sl].T).astype(bf16),
        })
    return in_maps


def assemble(results, bo):
    bo = np.asarray(bo, dtype=np.float32)
    out = np.empty((2, NTOK, DIM), np.float32)
    for be in range(2):
        acc = results[be * 4]["y"].astype(np.float32)
        for g in range(1, 4):
            acc += results[be * 4 + g]["y"].astype(np.float32)
        out[be] = acc + bo
    return out


def kernel(x1, x2, wq, wk, wv, wo, bo):
    nc = get_nc()
    in_maps = make_in_maps(x1, x2, wq, wk, wv, wo)
    last_err = None
    for attempt in range(3):
        try:
            res = bass_utils.run_bass_kernel_spmd(
                nc, in_maps, core_ids=list(range(8))
            )
            return assemble(res.results, bo)
        except Exception as e:  # transient NRT_EXEC_UNIT_UNRECOVERABLE etc.
            last_err = e
            import time as _time
            _time.sleep(5 * (attempt + 1))
    raise last_err


# revision 11
# speedup vs baseline: 1.5271x; 1.0455x over previous
"""Multi-head cross-attention (b=2, n=m=2048, dim=1024, 16 heads) on 8 trn2 cores.

Sharding: core = be*4 + g  (be = batch element, g = head group of 4 heads).
Each core computes, for its batch element and its 4 heads:
    Q^T = (wq_g @ x1^T), K^T = (wk_g @ x2^T), V = x2 @ wv_g^T
    S^T = K^T.T @ Q^T  (per head), P = exp(S * scale), O^T = [V | 1].T @ P
    (ones column = softmax denominator), normalize via reciprocal +
    partition broadcast, y_partial = O @ wo_g^T.
Host sums the 4 head-group partials per batch element and adds the bias.

HW facts this version exploits (measured on this axon/trn2 setup):
  - matmul with contraction dim 64 runs ~2.3x slower per row than C=128
    (437 vs 171 ns per 512-row bf16 matmul). The per-head S^T matmul
    (contraction = head_dim = 64) therefore uses the PACKED two-head K^T
    stationary (128 partitions) against a zero-padded per-head Q^T moving
    operand (the head's 64 rows hold Q^T, the other 64 rows are zero),
    computing the same S^T at full C=128 rate.
  - bf16 matmuls are ~1.27x faster than fp32r and halve SBUF/DMA traffic.
  - Engine queues execute IN ORDER, so emission order is the schedule.
    S^T pairs, the previous group's O accumulation, and out/Q-projection
    work are interleaved at ~1 slab granularity so no PE run between
    S-pairs exceeds the ~1us exp slab time.
  - The sync/SP queue (tile alloc/release bookkeeping + DMA issue) is a
    hidden serial resource: exp-output tiles are allocated 4 slabs at a
    time and x/weight loads are single strided DMAs to keep SP short.
"""

import sys

if "/opt/trn_rl_repo" not in sys.path:
    sys.path.insert(0, "/opt/trn_rl_repo")

import numpy as np

import concourse.tile as tile
from concourse import bacc, mybir
from concourse import bass_utils

P = 128
NTOK = 2048            # n = m = token count per batch element
DIM = 1024
HPC = 4                # heads per core
DH = 64                # head dim
HD = HPC * DH          # 256 = per-core projection width
ECH = DIM // P         # 8 contraction chunks
NCH = NTOK // 512      # 4 n-chunks of 512
MT = NTOK // P         # 16 m-tiles of 128
NG = NCH * HPC         # 16 (nq, h) groups
SCALE = DH ** -0.5
F32 = mybir.dt.float32
BF16 = mybir.dt.bfloat16

_CACHE: dict = {}


def _build(trace_sim: bool = False, repeat: int = 1):
    EXP = mybir.ActivationFunctionType.Exp
    nc = bacc.Bacc("TRN2", target_bir_lowering=False, debug=False, num_devices=8)
    x1T = nc.dram_tensor("x1t", [DIM, NTOK], BF16, kind="ExternalInput").ap()
    x2T = nc.dram_tensor("x2t", [DIM, NTOK], BF16, kind="ExternalInput").ap()
    wqT = nc.dram_tensor("wqt", [DIM, HD], BF16, kind="ExternalInput").ap()
    wkT = nc.dram_tensor("wkt", [DIM, HD], BF16, kind="ExternalInput").ap()
    wvT = nc.dram_tensor("wvt", [DIM, HD], BF16, kind="ExternalInput").ap()
    woT = nc.dram_tensor("wot", [HD, DIM], BF16, kind="ExternalInput").ap()
    y = nc.dram_tensor("y", [NTOK, DIM], BF16, kind="ExternalOutput").ap()

    x1T_s = x1T.rearrange("(po pi) n -> pi po n", pi=P)      # [128, 8, 2048]
    x2T_s = x2T.rearrange("(po pi) n -> pi po n", pi=P)
    wqT_r = wqT.rearrange("(po pi) m -> pi po m", pi=P)      # [128, 8, 256]
    wkT_r = wkT.rearrange("(po pi) m -> pi po m", pi=P)
    wvT_r = wvT.rearrange("(po pi) m -> pi po m", pi=P)
    woT_r = woT.rearrange("(po pi) e -> pi po e", pi=P)      # [128, 2, 1024]

    with tile.TileContext(nc, trace_sim=trace_sim) as tc:
      for _rep in range(repeat):
        with (
            tc.tile_pool(name="persist", bufs=1) as persist,
            tc.tile_pool(name="ps1", bufs=4, space="PSUM") as ps1,   # [128,512]
            tc.tile_pool(name="psS", bufs=2, space="PSUM") as psSp,  # [128,1024]
            tc.tile_pool(name="xq", bufs=2) as xqpool,
            tc.tile_pool(name="slab", bufs=24) as slabpool,
            tc.tile_pool(name="rec", bufs=2) as recpool,
            tc.tile_pool(name="bcp", bufs=2) as bcpool,
            tc.tile_pool(name="otmp", bufs=2) as tmppool,
            tc.tile_pool(name="ysb", bufs=2) as ypool,
        ):
            wq_sb = persist.tile([P, ECH, HD], BF16, tag="wq")
            wo_sb = persist.tile([P, 2, DIM], BF16, tag="wo")
            onesf = persist.tile([P, 64], BF16, tag="onesf")
            nc.vector.memset(onesf[:], 1.0)
            # warm the ACT exp table during initial DMAs
            dum = persist.tile([P, 8], F32, tag="dum")
            nc.vector.memset(dum[:], 0.0)
            nc.scalar.activation(dum[:], dum[:], EXP)
            # zero-padded per-head Q^T: head h occupies rows 64*(h%2)..+64 of
            # QTz[:, h, :]; the other 64 rows stay zero so the S^T matmul can
            # use the packed two-head K^T stationary at C=128.
            QTz = persist.tile([P, HPC, NTOK], BF16, tag="QTz")
            nc.vector.memset(QTz[:], 0.0)
            O_sb = persist.tile([P, 2, NTOK], BF16, tag="O")
            KT_sb = persist.tile([P, 2, NTOK], BF16, tag="KT")
            V_sb = persist.tile([P, MT, HPC, 65], BF16, tag="V")
            nc.vector.tensor_copy(
                V_sb[:, :, :, 64:65],
                onesf[:].rearrange("p (a b c) -> p a b c", a=MT, b=HPC, c=1),
            )

            slabs = {}

            def s_exp_pair(g, mtp):
                # S^T + exp for m-tiles (2*mtp, 2*mtp+1) of group g=(nq,h)
                nq, h = g // HPC, g % HPC
                pg = h // 2
                ns = slice(nq * 512, (nq + 1) * 512)
                psS = psSp.tile([P, 1024], F32, tag="psS", name=f"psS{g}_{mtp}")
                for sub in range(2):
                    mt = 2 * mtp + sub
                    nc.tensor.matmul(
                        psS[:, sub * 512:(sub + 1) * 512],
                        KT_sb[:, pg, mt * P:(mt + 1) * P],
                        QTz[:, h, ns],
                        start=True,
                        stop=True,
                    )
                es = slabpool.tile([P, 1024], BF16, tag="es", name=f"es{g}_{mtp}")
                slabs[(g, mtp)] = es
                nc.scalar.activation(es[:], psS[:], EXP, scale=SCALE)

            psO_t = {}

            def o_pair(g, mtp):
                # two O^T accumulation steps for group g (m-tiles 2mtp,2mtp+1)
                h = g % HPC
                if mtp == 0:
                    psO_t[g] = ps1.tile([P, 512], F32, tag="b1", name=f"psO{g}")
                psO = psO_t[g]
                es = slabs[(g, mtp)]
                for sub in range(2):
                    mt = 2 * mtp + sub
                    nc.tensor.matmul(
                        psO[0:65, :],
                        V_sb[:, mt, h, :],
                        es[:, sub * 512:(sub + 1) * 512],
                        start=(mt == 0),
                        stop=(mt == MT - 1),
                    )

            def normalize(g):
                nq, h = g // HPC, g % HPC
                pg, pos = h // 2, h % 2
                ns = slice(nq * 512, (nq + 1) * 512)
                psO = psO_t.pop(g)
                rec = recpool.tile([P, 512], F32, tag="rec")
                with nc.allow_low_precision(reason="softmax denom reciprocal"):
                    nc.vector.reciprocal(rec[64:65, :], psO[64:65, :])
                nc.sync.dma_start(rec[0:1, :], rec[64:65, :])
                bc = bcpool.tile([64, 512], F32, tag="bc")
                nc.gpsimd.partition_broadcast(bc[:], rec[0:1, :])
                if pos == 0:
                    nc.vector.tensor_mul(O_sb[0:64, pg, ns], psO[0:64, :], bc[:])
                else:
                    tmp = tmppool.tile([64, 512], BF16, tag="otmp")
                    nc.vector.tensor_mul(tmp[:], psO[0:64, :], bc[:])
                    nc.sync.dma_start(O_sb[64:128, pg, ns], tmp[:])

            # --- small PE work pieces, popped between slabs -----------------
            extra_q = []

            def qproj_pieces(nq):
                # Q^T projection for n-chunk nq, split into ~2-matmul pieces
                ns = slice(nq * 512, (nq + 1) * 512)
                state = {}

                def dma_piece():
                    xq = xqpool.tile([P, ECH, 512], BF16, tag="xq")
                    state["xq"] = xq
                    for ec in range(ECH):
                        nc.sync.dma_start(xq[:, ec], x1T_s[:, ec, ns])

                def mm_piece(pg, e0):
                    def run():
                        if e0 == 0:
                            state[pg] = ps1.tile(
                                [P, 512], F32, tag="b1", name=f"psq{nq}{pg}"
                            )
                        psq = state[pg]
                        for ec in (e0, e0 + 1):
                            nc.tensor.matmul(
                                psq[:],
                                wq_sb[:, ec, pg * P:(pg + 1) * P],
                                state["xq"][:, ec, :],
                                start=(ec == 0),
                                stop=(ec == ECH - 1),
                            )
                        if e0 == ECH - 2:
                            nc.vector.tensor_copy(
                                QTz[0:64, 2 * pg, ns], psq[0:64, :]
                            )
                            nc.vector.tensor_copy(
                                QTz[64:128, 2 * pg + 1, ns], psq[64:128, :]
                            )
                    return run

                yield dma_piece
                for pg in range(2):
                    for e0 in range(0, ECH, 2):
                        yield mm_piece(pg, e0)

            def outproj_pieces(nq):
                state = {}

                def mm_piece(nt, eo):
                    def run():
                        if eo == 0:
                            state[nt] = ypool.tile(
                                [P, DIM], BF16, tag="y", name=f"ysb{nt}"
                            )
                        psY = ps1.tile([P, 512], F32, tag="b1", name=f"psY{nt}{eo}")
                        for pg in range(2):
                            nc.tensor.matmul(
                                psY[:],
                                O_sb[:, pg, nt * P:(nt + 1) * P],
                                wo_sb[:, pg, eo * 512:(eo + 1) * 512],
                                start=(pg == 0),
                                stop=(pg == 1),
                            )
                        nc.vector.tensor_copy(
                            state[nt][:, eo * 512:(eo + 1) * 512], psY[:]
                        )
                        if eo == 1:
                            nc.gpsimd.dma_start(
                                y[nt * P:(nt + 1) * P, :], state[nt][:]
                            )
                    return run

                for nt in range(4 * nq, 4 * nq + 4):
                    for eo in range(2):
                        yield mm_piece(nt, eo)

            def pop_extra(k=1):
                for _ in range(k):
                    if extra_q:
                        extra_q.pop(0)()

            # ---- warmup: single x2 pass: K^T + V projections, with groups
            # 0 and 1 (nq=0, h=0/1) S^T+exp emitted as K chunks land ----
            with (
                tc.tile_pool(name="wkv", bufs=1) as wkvpool,
                tc.tile_pool(name="xk", bufs=2) as xkpool,
            ):
                wk_sb = wkvpool.tile([P, ECH, HD], BF16, tag="wk")
                for ec in range(ECH):
                    nc.sync.dma_start(wk_sb[:, ec], wkT_r[:, ec])
                wv_sb = wkvpool.tile([P, ECH, HD], BF16, tag="wv")

                for nq in range(NCH):
                    ns = slice(nq * 512, (nq + 1) * 512)
                    xk = xkpool.tile([P, ECH, 512], BF16, tag="xk")
                    for ec in range(ECH):
                        nc.sync.dma_start(xk[:, ec], x2T_s[:, ec, ns])
                    if nq == 0:
                        for ec in range(ECH):
                            nc.sync.dma_start(wv_sb[:, ec], wvT_r[:, ec])
                    for pg in range(2):
                        psq = ps1.tile([P, 512], F32, tag="b1", name=f"psk{nq}{pg}")
                        for ec in range(ECH):
                            nc.tensor.matmul(
                                psq[:],
                                wk_sb[:, ec, pg * P:(pg + 1) * P],
                                xk[:, ec, :],
                                start=(ec == 0),
                                stop=(ec == ECH - 1),
                            )
                        nc.vector.tensor_copy(KT_sb[:, pg, ns], psq[:])
                    # V for the 4 m-tiles covered by this x2 chunk
                    for sub in range(4):
                        mt = 4 * nq + sub
                        pv = ps1.tile([P, 512], F32, tag="b1", name=f"psv{mt}")
                        for ec in range(ECH):
                            nc.tensor.matmul(
                                pv[:, 0:256],
                                xk[:, ec, sub * P:(sub + 1) * P],
                                wv_sb[:, ec, :],
                                start=(ec == 0),
                                stop=(ec == ECH - 1),
                            )
                        nc.vector.tensor_copy(
                            V_sb[:, mt, :, 0:64],
                            pv[:, 0:256].rearrange("p (h d) -> p h d", d=64),
                        )
                    if nq == 0:
                        nc.sync.dma_start(wq_sb[:], wqT_r)
                        for piece in qproj_pieces(0):
                            piece()
                    if nq == NCH - 1:
                        nc.sync.dma_start(wo_sb[:], woT_r)
                    # warmup S/exp: groups 0,1 over the m-tiles this K chunk
                    # just produced
                    for h in range(2):
                        for mtp in (2 * nq, 2 * nq + 1):
                            s_exp_pair(h, mtp)

            # ---- main software pipeline over groups: S(idx) slabs are
            # interleaved with O(idx-1) pairs and small projection pieces ----
            for idx in range(1, NG):
                nq, h = idx // HPC, idx % HPC
                if h == 1 and nq + 1 < NCH:
                    extra_q.extend(qproj_pieces(nq + 1))
                if h == 2 and nq >= 1:
                    extra_q.extend(outproj_pieces(nq - 1))
                for mtp in range(MT // 2):
                    if idx >= 2:
                        s_exp_pair(idx, mtp)
                    o_pair(idx - 1, mtp)
                    pop_extra(1)
                normalize(idx - 1)
            for mtp in range(MT // 2):
                o_pair(NG - 1, mtp)
                pop_extra(2)
            normalize(NG - 1)
            while extra_q:
                pop_extra(1)
            for piece in outproj_pieces(NCH - 1):
                piece()
    nc.compile()
    return nc


def get_nc(trace_sim: bool = False, repeat: int = 1):
    key = ("nc", trace_sim, repeat)
    if key not in _CACHE:
        _CACHE[key] = _build(trace_sim, repeat)
    return _CACHE[key]


def make_in_maps(x1, x2, wq, wk, wv, wo):
    import ml_dtypes
    bf16 = ml_dtypes.bfloat16
    x1 = np.asarray(x1, dtype=np.float32)
    x2 = np.asarray(x2, dtype=np.float32)
    wq = np.asarray(wq, dtype=np.float32)
    wk = np.asarray(wk, dtype=np.float32)
    wv = np.asarray(wv, dtype=np.float32)
    wo = np.asarray(wo, dtype=np.float32)
    in_maps = []
    for core in range(8):
        be, g = core // 4, core % 4
        sl = slice(HD * g, HD * (g + 1))
        in_maps.append({
            "x1t": np.ascontiguousarray(x1[be].T).astype(bf16),
            "x2t": np.ascontiguousarray(x2[be].T).astype(bf16),
            "wqt": np.ascontiguousarray(wq[sl, :].T).astype(bf16),
            "wkt": np.ascontiguousarray(wk[sl, :].T).astype(bf16),
            "wvt": np.ascontiguousarray(wv[sl, :].T).astype(bf16),
            "wot": np.ascontiguousarray(wo[:, sl].T).astype(bf16),
        })
    return in_maps


def assemble(results, bo):
    bo = np.asarray(bo, dtype=np.float32)
    out = np.empty((2, NTOK, DIM), np.float32)
    for be in range(2):
        acc = results[be * 4]["y"].astype(np.float32)
        for g in range(1, 4):
            acc += results[be * 4 + g]["y"].astype(np.float32)
        out[be] = acc + bo
    return out


def kernel(x1, x2, wq, wk, wv, wo, bo):
    nc = get_nc()
    in_maps = make_in_maps(x1, x2, wq, wk, wv, wo)
    last_err = None
    for attempt in range(3):
        try:
            res = bass_utils.run_bass_kernel_spmd(
                nc, in_maps, core_ids=list(range(8))
            )
            return assemble(res.results, bo)
        except Exception as e:  # transient NRT_EXEC_UNIT_UNRECOVERABLE etc.
            last_err = e
            import time as _time
            _time.sleep(5 * (attempt + 1))
    raise last_err
